# revision 2
# baseline (speedup 1.0000x reference)
"""IsoMax pairwise-distance kernel for 8 TRN2 NeuronCores — fp8 DoubleRow.

Math:  out[b,m] = -|s| * sqrt(max(||xn_b||^2 + ||pn_m||^2 - 2*xn_b.pn_m, 0))
with xn/pn L2-normalized rows of x [4096,2048] and prototypes [12893,2048].
Since xn,pn are unit vectors this is -|s|*sqrt(2 - 2*cos). We compute
G = x_fp8 @ (64*pn)_fp8^T on the PE in DoubleRow fp8 mode (2 fp8 MACs/cell/
cycle, 256-deep contraction per matmul) and fuse the epilogue into one ACT
pass over all 4 PSUM banks: sqrt(svec[b]*G + 2s^2), svec = -2s^2/(64*||x_b||),
then a DVE negate (bf16 out).

Queue plan (engine queues are strict FIFO — emission order matters):
  gpsimd: x row loads only.  sync: x transposes + output DMA + p loads/
  transposes.  scalar: squares/sqrts + per-b-tile epilogue ACT.  vector:
  reciprocal/svec, fp8 casts, negate.  Per b-tile the epilogue/negate/out
  are emitted BEFORE the lookahead x_prep so they are not queued behind
  ops that wait on distant inputs.

Sharding: prototypes split across the 8 cores (output columns), x replicated.
M=12893 padded to 13312 = 8*1664. Inputs host-cast to bf16; output bf16,
host-cast back to f32.
"""

import os
import sys

sys.path.insert(0, "/opt/trn_rl_repo")

import numpy as np

INPUT_BF16 = True

B = 4096
D = 2048
M_FULL = 12893
N_CORES = 8
MC = 1664  # per-core prototype rows (13*128); 8*1664 = 13312 >= 12893
P = 128
KC = D // 256  # 8 DoubleRow contraction chunks of 256
MT = MC // P  # 13 m-tiles per core
BT = B // P  # 32 b-tiles

_cache = {}


def _build(s_abs: float, b_rows: int = B, mc: int = MC, repeat: int = 1):
    import concourse.bass as bass  # noqa: F401
    import concourse.mybir as mybir
    import concourse.tile as tile
    from concourse import bacc
    from contextlib import ExitStack

    f32 = mybir.dt.float32
    bf16 = mybir.dt.bfloat16
    f8 = mybir.dt.float8e4
    AF = mybir.ActivationFunctionType
    DR = mybir.MatmulPerfMode.DoubleRow
    kc = D // 256
    mt_n = mc // P
    bt_n = b_rows // P
    n_iter = bt_n * repeat
    two_s2 = 2.0 * s_abs * s_abs
    PSCALE = 64.0  # prototype pre-scale so unit rows land in fp8 normal range

    assert mc % 64 == 0
    cw = mc // 4  # psum chunk width (416 for mc=1664); 16B-aligned offsets
    chunks = [(ci * cw, cw) for ci in range(4)]

    nc = bacc.Bacc(None, target_bir_lowering=False)
    x_d = nc.dram_tensor("x", [b_rows, D], bf16, kind="ExternalInput")
    p_d = nc.dram_tensor("p", [mc, D], bf16, kind="ExternalInput")
    # output declared [b_rows, 4, cw] so device-side APs keep (4, cw) dims;
    # host reshapes back to [b_rows, mc]
    o_d = nc.dram_tensor("o", [b_rows, 4, cw], bf16, kind="ExternalOutput")

    with ExitStack() as ctx:
        tc = ctx.enter_context(tile.TileContext(nc))
        persist = ctx.enter_context(tc.tile_pool(name="persist", bufs=1))
        ppool = ctx.enter_context(tc.tile_pool(name="ppool", bufs=3))
        ptpool = ctx.enter_context(tc.tile_pool(name="ptpool", bufs=3))
        sq = ctx.enter_context(tc.tile_pool(name="sq", bufs=3))
        small = ctx.enter_context(tc.tile_pool(name="small", bufs=8))
        xpool = ctx.enter_context(tc.tile_pool(name="xpool", bufs=4))
        xtpool = ctx.enter_context(tc.tile_pool(name="xtpool", bufs=4))
        xt8pool = ctx.enter_context(tc.tile_pool(name="xt8pool", bufs=4))
        opool = ctx.enter_context(tc.tile_pool(name="opool", bufs=3))
        psum = ctx.enter_context(tc.tile_pool(name="psum", bufs=2, space="PSUM"))

        # prototypes, normalized*64, fp8, transposed: [d_inner, chunk, pair, m]
        pT8 = persist.tile([P, kc, 2, mc], f8)

        tiny_b = persist.tile([P, 1], f32, tag="tiny_b")
        nc.vector.memset(tiny_b, 1e-30)
        two_s2_b = persist.tile([P, 1], f32, tag="two_s2_b")
        nc.vector.memset(two_s2_b, two_s2)

        def x_prep(bt):
            x_bf = xpool.tile([P, D], bf16, tag="x_bf", name=f"x_bf_{bt}")
            nc.gpsimd.dma_start(x_bf, x_d[bt * P : (bt + 1) * P, :])
            xsq = sq.tile([P, D], bf16, tag="sq", name=f"xsq_{bt}")
            ssx = small.tile([P, 1], f32, tag="ss", name=f"ssx_{bt}")
            nc.scalar.activation(xsq, x_bf, AF.Square, accum_out=ssx)
            xnorm = small.tile([P, 1], f32, tag="nrm", name=f"xnorm_{bt}")
            nc.scalar.activation(xnorm, ssx, AF.Sqrt, bias=tiny_b)
            rx = small.tile([P, 1], f32, tag="rx", name=f"rx_{bt}")
            nc.vector.reciprocal(rx, xnorm)
            svec = small.tile([P, 1], f32, tag="svec", name=f"svec_{bt}")
            nc.vector.tensor_scalar_mul(svec, rx, -two_s2 / PSCALE)

            xT = xtpool.tile([P, kc, 2, P], bf16, tag="xT", name=f"xT_{bt}")
            nc.sync.dma_start(xT, x_bf, transpose=True)
            xT8 = xt8pool.tile([P, kc, 2, P], f8, tag="xT8", name=f"xT8_{bt}")
            nc.vector.tensor_scalar_mul(xT8, xT, 1.0)
            return xT8, svec

        # Hoist two b-tiles of x-prep so MMs can start as soon as the first
        # prototype chunk is ready; more would push p-prep's vector/scalar
        # work too far back in those FIFO queues.
        LOOKAHEAD = 2
        xprep = {}
        for bt_r in range(min(LOOKAHEAD, n_iter)):
            xprep[bt_r] = x_prep(bt_r % bt_n)

        for mt in range(mt_n):
            p_f = ppool.tile([P, D], bf16, tag="p_f")
            nc.sync.dma_start(p_f, p_d[mt * P : (mt + 1) * P, :])
            psq = sq.tile([P, D], bf16, tag="psq")
            ssp = small.tile([P, 1], f32, tag="ss", name=f"ssp_{mt}")
            nc.scalar.activation(psq, p_f, AF.Square, accum_out=ssp)
            # pnorm = ||p||/64  (scale folded into the Sqrt input)
            pnorm = small.tile([P, 1], f32, tag="nrm", name=f"pnorm_{mt}")
            nc.scalar.activation(
                pnorm, ssp, AF.Sqrt, bias=tiny_b, scale=1.0 / (PSCALE * PSCALE)
            )
            rp = small.tile([P, 1], f32, tag="rp")
            nc.vector.reciprocal(rp, pnorm)
            pn = ppool.tile([P, D], bf16, tag="pn")
            nc.vector.tensor_scalar_mul(pn, p_f, rp)
            pts = ptpool.tile([P, kc, 2, P], bf16, tag="pts")
            nc.sync.dma_start(pts, pn, transpose=True)
            nc.vector.tensor_scalar_mul(pT8[:, :, :, mt * P : (mt + 1) * P], pts, 1.0)

        for bt_r in range(n_iter):
            bt = bt_r % bt_n
            xT8, svec = xprep.pop(bt_r)

            ps = psum.tile([P, 4, 512], f32, tag="ps")
            # chunk-major: each psum bank's accumulation group completes in
            # kc consecutive MMs; chunk ci only needs m-tiles of its columns,
            # so the first b-tiles can start before p-prep fully finishes.
            for ci, (coff, w) in enumerate(chunks):
                for c in range(kc):
                    nc.tensor.matmul(
                        ps[:, ci, :w],
                        xT8[:, c],
                        pT8[:, c, :, coff : coff + w],
                        start=(c == 0),
                        stop=(c == kc - 1),
                        perf_mode=DR,
                    )
            t_sb = opool.tile([P, 4, cw], bf16, tag="t_sb")
            # sqrt(-2s^2/(64*||x||) * G + 2s^2) = s*sqrt(2 - 2*cos)
            nc.scalar.activation(
                t_sb, ps[:, :, :cw], AF.Sqrt, bias=two_s2_b, scale=svec
            )
            to = opool.tile([P, 4, cw], bf16, tag="to")
            nc.vector.tensor_scalar_mul(to, t_sb, -1.0)
            nc.sync.dma_start(o_d[bt * P : (bt + 1) * P], to)

            nxt = bt_r + LOOKAHEAD
            if nxt < n_iter:
                xprep[nxt] = x_prep(nxt % bt_n)

    nc.compile()
    return nc


LAST_RESULT = None


def _run(nc, in_maps, core_ids):
    from concourse import bass_utils

    global LAST_RESULT
    trace = bool(int(os.environ.get("ISOMAX_TRACE", "0")))
    LAST_RESULT = bass_utils.run_bass_kernel_spmd(
        nc, in_maps, core_ids=core_ids, trace=trace
    )
    return LAST_RESULT.results


def kernel(x, prototypes, distance_scale):
    import ml_dtypes

    bf = ml_dtypes.bfloat16
    x = np.asarray(x, dtype=np.float32)
    p = np.asarray(prototypes, dtype=np.float32)
    s_abs = float(abs(np.asarray(distance_scale).reshape(-1)[0].item()))
    m, d = p.shape
    assert (m, d) == (M_FULL, D) and x.shape == (B, D)

    key = ("full", s_abs)
    if key not in _cache:
        _cache[key] = _build(s_abs)
    nc = _cache[key]

    x_bf = np.ascontiguousarray(x.astype(bf))
    p_pad = np.zeros((N_CORES * MC, D), bf)
    p_pad[:m] = p.astype(bf)
    in_maps = [
        {"x": x_bf, "p": np.ascontiguousarray(p_pad[i * MC : (i + 1) * MC])}
        for i in range(N_CORES)
    ]
    results = _run(nc, in_maps, list(range(N_CORES)))
    out = np.concatenate(
        [results[i]["o"].reshape(B, MC) for i in range(N_CORES)], axis=1
    )
    return np.ascontiguousarray(out[:, :m]).astype(np.float32)


# revision 3
# speedup vs baseline: 1.0193x; 1.0193x over previous
"""IsoMax pairwise-distance kernel for 8 TRN2 NeuronCores — fp8 DoubleRow.

Math:  out[b,m] = -|s| * sqrt(max(||xn_b||^2 + ||pn_m||^2 - 2*xn_b.pn_m, 0))
with xn/pn L2-normalized rows of x [4096,2048] and prototypes [12893,2048].
Since xn,pn are unit vectors this is -|s|*sqrt(2 - 2*cos). We compute
G = x_fp8 @ (64*pn)_fp8^T on the PE in DoubleRow fp8 mode (2 fp8 MACs/cell/
cycle, 256-deep contraction per matmul) and fuse the epilogue into one ACT
pass over all 4 PSUM banks: sqrt(svec[b]*G + 2s^2), svec = -2s^2/(64*||x_b||),
then a DVE negate (bf16 out).

Queue plan (engine queues are strict FIFO — emission order matters):
  gpsimd: x row loads only.  sync: x transposes + output DMA + p loads/
  transposes.  scalar: squares/sqrts + per-b-tile epilogue ACT.  vector:
  reciprocal/svec, fp8 casts, negate.  Per b-tile the epilogue/negate/out
  are emitted BEFORE the lookahead x_prep so they are not queued behind
  ops that wait on distant inputs.

Sharding: prototypes split across the 8 cores (output columns), x replicated.
M=12893 padded to 13312 = 8*1664. Inputs host-cast to bf16; output bf16,
host-cast back to f32.
"""

import os
import sys

sys.path.insert(0, "/opt/trn_rl_repo")

import numpy as np

INPUT_BF16 = True

B = 4096
D = 2048
M_FULL = 12893
N_CORES = 8
MC = 1664  # per-core prototype rows (13*128); 8*1664 = 13312 >= 12893
P = 128
KC = D // 256  # 8 DoubleRow contraction chunks of 256
MT = MC // P  # 13 m-tiles per core
BT = B // P  # 32 b-tiles

_cache = {}


def _build(s_abs: float, b_rows: int = B, mc: int = MC, repeat: int = 1):
    import concourse.bass as bass  # noqa: F401
    import concourse.mybir as mybir
    import concourse.tile as tile
    from concourse import bacc
    from contextlib import ExitStack

    f32 = mybir.dt.float32
    bf16 = mybir.dt.bfloat16
    f8 = mybir.dt.float8e4
    AF = mybir.ActivationFunctionType
    DR = mybir.MatmulPerfMode.DoubleRow
    kc = D // 256
    mt_n = mc // P
    bt_n = b_rows // P
    n_iter = bt_n * repeat
    two_s2 = 2.0 * s_abs * s_abs
    PSCALE = 64.0  # prototype pre-scale so unit rows land in fp8 normal range

    assert mc % 64 == 0
    cw = mc // 4  # psum chunk width (416 for mc=1664); 16B-aligned offsets
    chunks = [(ci * cw, cw) for ci in range(4)]

    nc = bacc.Bacc(None, target_bir_lowering=False)
    x_d = nc.dram_tensor("x", [b_rows, D], bf16, kind="ExternalInput")
    p_d = nc.dram_tensor("p", [mc, D], bf16, kind="ExternalInput")
    # output declared [b_rows, 4, cw] so device-side APs keep (4, cw) dims;
    # host reshapes back to [b_rows, mc]
    o_d = nc.dram_tensor("o", [b_rows, 4, cw], bf16, kind="ExternalOutput")

    with ExitStack() as ctx:
        tc = ctx.enter_context(tile.TileContext(nc))
        persist = ctx.enter_context(tc.tile_pool(name="persist", bufs=1))
        ppool = ctx.enter_context(tc.tile_pool(name="ppool", bufs=3))
        plpool = ctx.enter_context(tc.tile_pool(name="plpool", bufs=mt_n))
        ptpool = ctx.enter_context(tc.tile_pool(name="ptpool", bufs=3))
        sq = ctx.enter_context(tc.tile_pool(name="sq", bufs=3))
        small = ctx.enter_context(tc.tile_pool(name="small", bufs=8))
        xpool = ctx.enter_context(tc.tile_pool(name="xpool", bufs=4))
        xtpool = ctx.enter_context(tc.tile_pool(name="xtpool", bufs=4))
        xt8pool = ctx.enter_context(tc.tile_pool(name="xt8pool", bufs=4))
        opool = ctx.enter_context(tc.tile_pool(name="opool", bufs=3))
        psum = ctx.enter_context(tc.tile_pool(name="psum", bufs=2, space="PSUM"))

        # prototypes, normalized*64, fp8, transposed: [d_inner, chunk, pair, m]
        pT8 = persist.tile([P, kc, 2, mc], f8)

        tiny_b = persist.tile([P, 1], f32, tag="tiny_b")
        nc.vector.memset(tiny_b, 1e-30)
        two_s2_b = persist.tile([P, 1], f32, tag="two_s2_b")
        nc.vector.memset(two_s2_b, two_s2)

        def x_prep(bt):
            x_bf = xpool.tile([P, D], bf16, tag="x_bf", name=f"x_bf_{bt}")
            nc.gpsimd.dma_start(x_bf, x_d[bt * P : (bt + 1) * P, :])
            xsq = sq.tile([P, D], bf16, tag="sq", name=f"xsq_{bt}")
            ssx = small.tile([P, 1], f32, tag="ss", name=f"ssx_{bt}")
            nc.scalar.activation(xsq, x_bf, AF.Square, accum_out=ssx)
            xnorm = small.tile([P, 1], f32, tag="nrm", name=f"xnorm_{bt}")
            nc.scalar.activation(xnorm, ssx, AF.Sqrt, bias=tiny_b)
            rx = small.tile([P, 1], f32, tag="rx", name=f"rx_{bt}")
            nc.vector.reciprocal(rx, xnorm)
            svec = small.tile([P, 1], f32, tag="svec", name=f"svec_{bt}")
            nc.vector.tensor_scalar_mul(svec, rx, -two_s2 / PSCALE)

            xT = xtpool.tile([P, kc, 2, P], bf16, tag="xT", name=f"xT_{bt}")
            nc.sync.dma_start(xT, x_bf, transpose=True)
            xT8 = xt8pool.tile([P, kc, 2, P], f8, tag="xT8", name=f"xT8_{bt}")
            nc.vector.tensor_scalar_mul(xT8, xT, 1.0)
            return xT8, svec

        # Hoist two b-tiles of x-prep so MMs can start as soon as the first
        # prototype chunk is ready; more would push p-prep's vector/scalar
        # work too far back in those FIFO queues.
        LOOKAHEAD = 2
        xprep = {}
        for bt_r in range(min(LOOKAHEAD, n_iter)):
            xprep[bt_r] = x_prep(bt_r % bt_n)

        # all prototype loads up front: a load must never queue behind the
        # previous m-tile's transpose (which waits on that tile's norm chain)
        p_fs = []
        for mt in range(mt_n):
            p_f = plpool.tile([P, D], bf16, tag="p_f", name=f"p_f_{mt}")
            nc.sync.dma_start(p_f, p_d[mt * P : (mt + 1) * P, :])
            p_fs.append(p_f)

        for mt in range(mt_n):
            p_f = p_fs[mt]
            psq = sq.tile([P, D], bf16, tag="psq")
            ssp = small.tile([P, 1], f32, tag="ss", name=f"ssp_{mt}")
            nc.scalar.activation(psq, p_f, AF.Square, accum_out=ssp)
            # pnorm = ||p||/64  (scale folded into the Sqrt input)
            pnorm = small.tile([P, 1], f32, tag="nrm", name=f"pnorm_{mt}")
            nc.scalar.activation(
                pnorm, ssp, AF.Sqrt, bias=tiny_b, scale=1.0 / (PSCALE * PSCALE)
            )
            rp = small.tile([P, 1], f32, tag="rp")
            nc.vector.reciprocal(rp, pnorm)
            pn = ppool.tile([P, D], bf16, tag="pn")
            nc.vector.tensor_scalar_mul(pn, p_f, rp)
            pts = ptpool.tile([P, kc, 2, P], bf16, tag="pts")
            nc.sync.dma_start(pts, pn, transpose=True)
            nc.vector.tensor_scalar_mul(pT8[:, :, :, mt * P : (mt + 1) * P], pts, 1.0)

        for bt_r in range(n_iter):
            bt = bt_r % bt_n
            xT8, svec = xprep.pop(bt_r)

            ps = psum.tile([P, 4, 512], f32, tag="ps")
            # chunk-major: each psum bank's accumulation group completes in
            # kc consecutive MMs; chunk ci only needs m-tiles of its columns,
            # so the first b-tiles can start before p-prep fully finishes.
            for ci, (coff, w) in enumerate(chunks):
                for c in range(kc):
                    nc.tensor.matmul(
                        ps[:, ci, :w],
                        xT8[:, c],
                        pT8[:, c, :, coff : coff + w],
                        start=(c == 0),
                        stop=(c == kc - 1),
                        perf_mode=DR,
                    )
            t_sb = opool.tile([P, 4, cw], bf16, tag="t_sb")
            # sqrt(-2s^2/(64*||x||) * G + 2s^2) = s*sqrt(2 - 2*cos)
            nc.scalar.activation(
                t_sb, ps[:, :, :cw], AF.Sqrt, bias=two_s2_b, scale=svec
            )
            to = opool.tile([P, 4, cw], bf16, tag="to")
            nc.vector.tensor_scalar_mul(to, t_sb, -1.0)
            nc.sync.dma_start(o_d[bt * P : (bt + 1) * P], to)

            nxt = bt_r + LOOKAHEAD
            if nxt < n_iter:
                xprep[nxt] = x_prep(nxt % bt_n)

    nc.compile()
    return nc


LAST_RESULT = None


def _run(nc, in_maps, core_ids):
    from concourse import bass_utils

    global LAST_RESULT
    trace = bool(int(os.environ.get("ISOMAX_TRACE", "0")))
    LAST_RESULT = bass_utils.run_bass_kernel_spmd(
        nc, in_maps, core_ids=core_ids, trace=trace
    )
    return LAST_RESULT.results


def kernel(x, prototypes, distance_scale):
    import ml_dtypes

    bf = ml_dtypes.bfloat16
    x = np.asarray(x, dtype=np.float32)
    p = np.asarray(prototypes, dtype=np.float32)
    s_abs = float(abs(np.asarray(distance_scale).reshape(-1)[0].item()))
    m, d = p.shape
    assert (m, d) == (M_FULL, D) and x.shape == (B, D)

    key = ("full", s_abs)
    if key not in _cache:
        _cache[key] = _build(s_abs)
    nc = _cache[key]

    x_bf = np.ascontiguousarray(x.astype(bf))
    p_pad = np.zeros((N_CORES * MC, D), bf)
    p_pad[:m] = p.astype(bf)
    in_maps = [
        {"x": x_bf, "p": np.ascontiguousarray(p_pad[i * MC : (i + 1) * MC])}
        for i in range(N_CORES)
    ]
    results = _run(nc, in_maps, list(range(N_CORES)))
    out = np.concatenate(
        [results[i]["o"].reshape(B, MC) for i in range(N_CORES)], axis=1
    )
    return np.ascontiguousarray(out[:, :m]).astype(np.float32)


# revision 4
# speedup vs baseline: 1.0209x; 1.0016x over previous
"""IsoMax pairwise-distance kernel for 8 TRN2 NeuronCores — fp8 DoubleRow.

Math:  out[b,m] = -|s| * sqrt(max(||xn_b||^2 + ||pn_m||^2 - 2*xn_b.pn_m, 0))
with xn/pn L2-normalized rows of x [4096,2048] and prototypes [12893,2048].
Since xn,pn are unit vectors this is -|s|*sqrt(2 - 2*cos). We compute
G = x_fp8 @ (64*pn)_fp8^T on the PE in DoubleRow fp8 mode (2 fp8 MACs/cell/
cycle, 256-deep contraction per matmul) and fuse the epilogue into one ACT
pass over all 4 PSUM banks: sqrt(svec[b]*G + 2s^2), svec = -2s^2/(64*||x_b||),
then a DVE negate (bf16 out).

Queue plan (engine queues are strict FIFO — emission order matters):
  gpsimd: x row loads only.  sync: x transposes + output DMA + p loads/
  transposes.  scalar: squares/sqrts + per-b-tile epilogue ACT.  vector:
  reciprocal/svec, fp8 casts, negate.  Per b-tile the epilogue/negate/out
  are emitted BEFORE the lookahead x_prep so they are not queued behind
  ops that wait on distant inputs.

Sharding: prototypes split across the 8 cores (output columns), x replicated.
M=12893 padded to 13312 = 8*1664. Inputs host-cast to bf16; output bf16,
host-cast back to f32.
"""

import os
import sys

sys.path.insert(0, "/opt/trn_rl_repo")

import numpy as np

INPUT_BF16 = True

B = 4096
D = 2048
M_FULL = 12893
N_CORES = 8
MC = 1664  # per-core prototype rows (13*128); 8*1664 = 13312 >= 12893
P = 128
KC = D // 256  # 8 DoubleRow contraction chunks of 256
MT = MC // P  # 13 m-tiles per core
BT = B // P  # 32 b-tiles

_cache = {}


def _build(s_abs: float, b_rows: int = B, mc: int = MC, repeat: int = 1):
    import concourse.bass as bass  # noqa: F401
    import concourse.mybir as mybir
    import concourse.tile as tile
    from concourse import bacc
    from contextlib import ExitStack

    f32 = mybir.dt.float32
    bf16 = mybir.dt.bfloat16
    f8 = mybir.dt.float8e4
    AF = mybir.ActivationFunctionType
    DR = mybir.MatmulPerfMode.DoubleRow
    kc = D // 256
    mt_n = mc // P
    bt_n = b_rows // P
    n_iter = bt_n * repeat
    two_s2 = 2.0 * s_abs * s_abs
    PSCALE = 64.0  # prototype pre-scale so unit rows land in fp8 normal range

    assert mc % 64 == 0
    cw = mc // 4  # psum chunk width (416 for mc=1664); 16B-aligned offsets
    chunks = [(ci * cw, cw) for ci in range(4)]

    nc = bacc.Bacc(None, target_bir_lowering=False)
    x_d = nc.dram_tensor("x", [b_rows, D], bf16, kind="ExternalInput")
    p_d = nc.dram_tensor("p", [mc, D], bf16, kind="ExternalInput")
    # output declared [b_rows, 4, cw] so device-side APs keep (4, cw) dims;
    # host reshapes back to [b_rows, mc]
    o_d = nc.dram_tensor("o", [b_rows, 4, cw], bf16, kind="ExternalOutput")

    with ExitStack() as ctx:
        tc = ctx.enter_context(tile.TileContext(nc))
        persist = ctx.enter_context(tc.tile_pool(name="persist", bufs=1))
        ppool = ctx.enter_context(tc.tile_pool(name="ppool", bufs=3))
        plpool = ctx.enter_context(tc.tile_pool(name="plpool", bufs=mt_n))
        ptpool = ctx.enter_context(tc.tile_pool(name="ptpool", bufs=3))
        sq = ctx.enter_context(tc.tile_pool(name="sq", bufs=3))
        small = ctx.enter_context(tc.tile_pool(name="small", bufs=8))
        xpool = ctx.enter_context(tc.tile_pool(name="xpool", bufs=5))
        xtpool = ctx.enter_context(tc.tile_pool(name="xtpool", bufs=5))
        xt8pool = ctx.enter_context(tc.tile_pool(name="xt8pool", bufs=5))
        opool = ctx.enter_context(tc.tile_pool(name="opool", bufs=3))
        psum = ctx.enter_context(tc.tile_pool(name="psum", bufs=2, space="PSUM"))

        # prototypes, normalized*64, fp8, transposed: [d_inner, chunk, pair, m]
        pT8 = persist.tile([P, kc, 2, mc], f8)

        tiny_b = persist.tile([P, 1], f32, tag="tiny_b")
        nc.vector.memset(tiny_b, 1e-30)
        two_s2_b = persist.tile([P, 1], f32, tag="two_s2_b")
        nc.vector.memset(two_s2_b, two_s2)

        def x_prep(bt):
            x_bf = xpool.tile([P, D], bf16, tag="x_bf", name=f"x_bf_{bt}")
            nc.gpsimd.dma_start(x_bf, x_d[bt * P : (bt + 1) * P, :])
            xsq = sq.tile([P, D], bf16, tag="sq", name=f"xsq_{bt}")
            ssx = small.tile([P, 1], f32, tag="ss", name=f"ssx_{bt}")
            nc.scalar.activation(xsq, x_bf, AF.Square, accum_out=ssx)
            xnorm = small.tile([P, 1], f32, tag="nrm", name=f"xnorm_{bt}")
            nc.scalar.activation(xnorm, ssx, AF.Sqrt, bias=tiny_b)
            rx = small.tile([P, 1], f32, tag="rx", name=f"rx_{bt}")
            nc.vector.reciprocal(rx, xnorm)
            svec = small.tile([P, 1], f32, tag="svec", name=f"svec_{bt}")
            nc.vector.tensor_scalar_mul(svec, rx, -two_s2 / PSCALE)

            xT = xtpool.tile([P, kc, 2, P], bf16, tag="xT", name=f"xT_{bt}")
            nc.sync.dma_start(xT, x_bf, transpose=True)
            xT8 = xt8pool.tile([P, kc, 2, P], f8, tag="xT8", name=f"xT8_{bt}")
            nc.vector.tensor_scalar_mul(xT8, xT, 1.0)
            return xT8, svec

        # Hoist two b-tiles of x-prep so MMs can start as soon as the first
        # prototype chunk is ready; more would push p-prep's vector/scalar
        # work too far back in those FIFO queues.
        LOOKAHEAD = 3
        xprep = {}
        for bt_r in range(min(LOOKAHEAD, n_iter)):
            xprep[bt_r] = x_prep(bt_r % bt_n)

        # all prototype loads up front: a load must never queue behind the
        # previous m-tile's transpose (which waits on that tile's norm chain)
        p_fs = []
        for mt in range(mt_n):
            p_f = plpool.tile([P, D], bf16, tag="p_f", name=f"p_f_{mt}")
            nc.sync.dma_start(p_f, p_d[mt * P : (mt + 1) * P, :])
            p_fs.append(p_f)

        for mt in range(mt_n):
            p_f = p_fs[mt]
            psq = sq.tile([P, D], bf16, tag="psq")
            ssp = small.tile([P, 1], f32, tag="ss", name=f"ssp_{mt}")
            nc.scalar.activation(psq, p_f, AF.Square, accum_out=ssp)
            # pnorm = ||p||/64  (scale folded into the Sqrt input)
            pnorm = small.tile([P, 1], f32, tag="nrm", name=f"pnorm_{mt}")
            nc.scalar.activation(
                pnorm, ssp, AF.Sqrt, bias=tiny_b, scale=1.0 / (PSCALE * PSCALE)
            )
            rp = small.tile([P, 1], f32, tag="rp")
            nc.vector.reciprocal(rp, pnorm)
            pn = ppool.tile([P, D], bf16, tag="pn")
            nc.vector.tensor_scalar_mul(pn, p_f, rp)
            pts = ptpool.tile([P, kc, 2, P], bf16, tag="pts")
            nc.sync.dma_start(pts, pn, transpose=True)
            nc.vector.tensor_scalar_mul(pT8[:, :, :, mt * P : (mt + 1) * P], pts, 1.0)

        for bt_r in range(n_iter):
            bt = bt_r % bt_n
            xT8, svec = xprep.pop(bt_r)

            ps = psum.tile([P, 4, 512], f32, tag="ps")
            # chunk-major: each psum bank's accumulation group completes in
            # kc consecutive MMs; chunk ci only needs m-tiles of its columns,
            # so the first b-tiles can start before p-prep fully finishes.
            for ci, (coff, w) in enumerate(chunks):
                for c in range(kc):
                    nc.tensor.matmul(
                        ps[:, ci, :w],
                        xT8[:, c],
                        pT8[:, c, :, coff : coff + w],
                        start=(c == 0),
                        stop=(c == kc - 1),
                        perf_mode=DR,
                    )
            t_sb = opool.tile([P, 4, cw], bf16, tag="t_sb")
            # sqrt(-2s^2/(64*||x||) * G + 2s^2) = s*sqrt(2 - 2*cos)
            nc.scalar.activation(
                t_sb, ps[:, :, :cw], AF.Sqrt, bias=two_s2_b, scale=svec
            )
            to = opool.tile([P, 4, cw], bf16, tag="to")
            nc.vector.tensor_scalar_mul(to, t_sb, -1.0)
            nc.sync.dma_start(o_d[bt * P : (bt + 1) * P], to)

            nxt = bt_r + LOOKAHEAD
            if nxt < n_iter:
                xprep[nxt] = x_prep(nxt % bt_n)

    nc.compile()
    return nc


LAST_RESULT = None


def _run(nc, in_maps, core_ids):
    from concourse import bass_utils

    global LAST_RESULT
    trace = bool(int(os.environ.get("ISOMAX_TRACE", "0")))
    LAST_RESULT = bass_utils.run_bass_kernel_spmd(
        nc, in_maps, core_ids=core_ids, trace=trace
    )
    return LAST_RESULT.results


def kernel(x, prototypes, distance_scale):
    import ml_dtypes

    bf = ml_dtypes.bfloat16
    x = np.asarray(x, dtype=np.float32)
    p = np.asarray(prototypes, dtype=np.float32)
    s_abs = float(abs(np.asarray(distance_scale).reshape(-1)[0].item()))
    m, d = p.shape
    assert (m, d) == (M_FULL, D) and x.shape == (B, D)

    key = ("full", s_abs)
    if key not in _cache:
        _cache[key] = _build(s_abs)
    nc = _cache[key]

    x_bf = np.ascontiguousarray(x.astype(bf))
    p_pad = np.zeros((N_CORES * MC, D), bf)
    p_pad[:m] = p.astype(bf)
    in_maps = [
        {"x": x_bf, "p": np.ascontiguousarray(p_pad[i * MC : (i + 1) * MC])}
        for i in range(N_CORES)
    ]
    results = _run(nc, in_maps, list(range(N_CORES)))
    out = np.concatenate(
        [results[i]["o"].reshape(B, MC) for i in range(N_CORES)], axis=1
    )
    return np.ascontiguousarray(out[:, :m]).astype(np.float32)
